# revision 1
# baseline (speedup 1.0000x reference)
# Multi-head masked attention (V = Q source quirk; Wv unused) on 8 TRN2 NeuronCores.
#
# Sharding: 8 cores = 4 batches x 2 query-parities. Core c handles batch b = c//2
# and the interleaved query tiles {p, p+2, p+4, ...} (p = c%2) of that batch, for
# ALL 16 heads. Each core projects K^T (all keys), Q-natural (all positions; it is
# also V due to the V=Q source bug), and Q^T for its own query half, runs causal
# attention, and produces its (disjoint) half of the output rows including the
# final projection + bias. No collectives needed; host reassembles rows.
#
# Layouts (per core, bf16 matmul operands, fp32 PSUM accumulation):
#   kT  [128=d-in-pair, HP, S]    scores lhsT  (head even: partitions 0-63)
#   qT  [128=d-in-pair, HP, Lq]   scores rhs
#   qn  [128=k-in-tile, S/128, H*(D+1)]  attnV lhsT; col D of each head slot is
#                                 a ones column -> PSUM partition 64 accumulates
#                                 the softmax denominator for free.
#   scores computed transposed (scoresT[k, q] = K @ Q^T) so the softmax sum over
#   keys is a partition-dim reduction, done by the ones column on the PE.
#   Causal masking: column-trimmed matmul/exp ranges + one data-driven frontier
#   mask multiply per (chunk, k-tile) unit; masks are per-core input data so the
#   program is SPMD-uniform across parities. Units with t < chunk start are
#   always fully unmasked (both parities) and skip the multiply.
#
# Perf structure: PE warmup matmuls keep the HAM clock-gate at 8/8 during the
# initial x/weight DMA; normalization is inlined per (hp, qc) so the PE never
# idles into the output projection; weight pools are double-buffered.

import sys

for _p in ("/opt/trn_rl_repo",):
    if _p not in sys.path:
        sys.path.append(_p)

import numpy as np
import ml_dtypes

BF16 = ml_dtypes.bfloat16

B, S, E, H = 4, 2048, 1024, 16
D = E // H
NCORES = 8

_CACHE = {}


def _unit_list(S, P=128):
    """Attention inner-loop units (qc, t, jloc) shared by builder and host."""
    Lq = S // 2
    CH = min(512, Lq)
    spc = CH // P
    n_ch = Lq // CH
    units = []
    for qc in range(n_ch):
        t_max = 2 * (qc * spc + spc - 1) + 1
        for t in range(t_max + 1):
            jstar = t // 2  # first possibly-valid local subtile (parity-1 basis)
            jloc = max(0, jstar - qc * spc)
            units.append((qc, t, jloc))
    return units, CH, spc, n_ch


def _masked_units(S, P=128):
    """Units whose frontier 128-block needs the data-driven mask multiply.

    For t < 2*qc*spc the whole unit is strictly below the causal frontier for
    BOTH parities (g(sub0) = 2*qc*spc + parity > t), so the mask is all-ones
    and the multiply is skipped. Returns {(qc, t): mask_index}.
    """
    units, CH, spc, n_ch = _unit_list(S, P)
    mi = {}
    for qc, t, jloc in units:
        if t >= 2 * qc * spc:
            mi[(qc, t)] = len(mi)
    return mi


def _build_program(S, E, H, n_cores=NCORES):
    import concourse.bass as bass
    import concourse.mybir as mybir
    import concourse.tile as tile
    from concourse import bacc
    from contextlib import ExitStack

    P = 128
    D = E // H
    assert D == 64 and S % 256 == 0 and E % P == 0
    S_t = S // P          # global seq tiles
    nq = S_t // 2         # local q tiles
    Lq = S // 2           # local q length
    E_t = E // P
    HP = H // 2           # head pairs
    DCH = min(512, E)     # projection d-chunk
    SCH = min(512, S)     # projection s-chunk
    QCH = min(512, Lq)    # projection q-chunk
    ECH = min(512, E)     # outproj e-chunk
    units, CH, spc, n_ch = _unit_list(S, P)
    CHB = max(CH, 512)    # per-head scores region: always a full PSUM bank
    mu = _masked_units(S, P)
    U2 = len(mu)
    f32 = mybir.dt.float32
    bf16 = mybir.dt.bfloat16
    Exp = mybir.ActivationFunctionType.Exp
    scale = 1.0 / float(np.sqrt(E))

    uofs = {}
    for ui, (qc, t, jloc) in enumerate(units):
        uofs[(qc, t)] = jloc

    nc = bacc.Bacc(
        "TRN2", target_bir_lowering=False, debug=False, num_devices=n_cores
    )

    xT_d = nc.dram_tensor("xT", [E, S], bf16, kind="ExternalInput").ap()
    xqT_d = nc.dram_tensor("xqT", [E, Lq], bf16, kind="ExternalInput").ap()
    wqT_d = nc.dram_tensor("wqT", [E, E], bf16, kind="ExternalInput").ap()
    wkT_d = nc.dram_tensor("wkT", [E, E], bf16, kind="ExternalInput").ap()
    woT_d = nc.dram_tensor("woT", [E, E], bf16, kind="ExternalInput").ap()
    bo_d = nc.dram_tensor("bo", [1, E], bf16, kind="ExternalInput").ap()
    # masks stored host-side already partition-major: [P, U2, P]
    mask_d = nc.dram_tensor("masks", [P, U2 * P], bf16, kind="ExternalInput").ap()
    out_d = nc.dram_tensor("out", [Lq, E], bf16, kind="ExternalOutput").ap()

    with tile.TileContext(nc) as tc, ExitStack() as ctx:
        main = ctx.enter_context(tc.tile_pool(name="main", bufs=1))
        expp = ctx.enter_context(tc.tile_pool(name="expp", bufs=3))
        stgp = ctx.enter_context(tc.tile_pool(name="stgp", bufs=2))
        ostp = ctx.enter_context(tc.tile_pool(name="ostp", bufs=2))

        kT = main.tile([P, HP, S], bf16)
        qT = main.tile([P, HP, Lq], bf16)
        qn = main.tile([P, S_t, H * (D + 1)], bf16)
        attnT = main.tile([P, HP, Lq], bf16)
        masks = main.tile([P, U2, P], bf16)
        ones128 = main.tile([1, P], bf16)
        bo_sb = main.tile([1, E], bf16)
        wsrc = main.tile([P, 256], bf16)
        wo = main.tile([P, HP, E], bf16)

        nc.vector.memset(ones128, 1.0)
        nc.vector.memset(wsrc, 0.0)

        # ---- PE warmup: keep HAM at 8/8 while the initial DMAs stream ----
        with tc.tile_pool(name="wps", bufs=1, space="PSUM") as wpsp:
            wdst = wpsp.tile([P, 256], f32)
            for _ in range(112):
                nc.tensor.matmul(wdst, wsrc[:, 0:P], wsrc, start=True, stop=True)

        qn4 = qn.rearrange("p t (h c) -> p t h c", c=D + 1)

        with tc.tile_pool(name="ph1", bufs=1) as ph1, \
                tc.tile_pool(name="whp", bufs=2) as whp:
            xT_r = xT_d.rearrange("(t p) s -> p t s", p=P)
            xqT_r = xqT_d.rearrange("(t p) s -> p t s", p=P)
            wq_r = wqT_d.rearrange("(t p) d -> p t d", p=P)
            wk_r = wkT_d.rearrange("(t p) d -> p t d", p=P)
            wo_r = woT_d.rearrange("(t p) e -> p t e", p=P)

            # DMA issue order is sync-queue program order; front-load the
            # tensors the first matmuls need: xT, then wqf chunk 0, then the
            # rest. masks/bo are needed much later.
            xTs, xqTs = [], []
            for e in range(E_t):
                xe = ph1.tile([P, S], bf16, tag=f"xT{e}")
                nc.sync.dma_start(out=xe, in_=xT_r[:, e, :])
                xTs.append(xe)

            with tc.tile_pool(name="wqf", bufs=2) as wqfp:
                nh = DCH // D  # heads per d-chunk
                wqf_all = []
                for dc in range(E // DCH):
                    wqfs = []
                    for e in range(E_t):
                        we = wqfp.tile([P, DCH], bf16, tag=f"wqf{e}")
                        nc.sync.dma_start(
                            out=we, in_=wq_r[:, e, dc * DCH:(dc + 1) * DCH]
                        )
                        wqfs.append(we)
                    wqf_all.append(wqfs)
                    if dc == 0:
                        # lower-priority loads issue after wqf chunk 0
                        for e in range(E_t):
                            xqe = ph1.tile([P, Lq], bf16, tag=f"xqT{e}")
                            nc.sync.dma_start(out=xqe, in_=xqT_r[:, e, :])
                            xqTs.append(xqe)

                nc.sync.dma_start(
                    out=masks.rearrange("p u m -> p (u m)"), in_=mask_d
                )
                nc.sync.dma_start(out=bo_sb, in_=bo_d)
                for cp in range(HP):
                    nc.sync.dma_start(out=wo[:, cp, :], in_=wo_r[:, cp, :])

                # ---- Q natural (= V) projection, head-padded with ones ----
                with tc.tile_pool(name="pq", bufs=2, space="PSUM") as pqp:
                    for dc in range(E // DCH):
                        wqfs = wqf_all[dc]
                        for st in range(S_t):
                            ps = pqp.tile([P, DCH], f32, tag="ps", name="ps")
                            for e in range(E_t):
                                nc.tensor.matmul(
                                    ps,
                                    xTs[e][:, st * P:(st + 1) * P],
                                    wqfs[e],
                                    start=(e == 0),
                                    stop=(e == E_t - 1),
                                )
                            nc.vector.tensor_copy(
                                out=qn4[:, st, dc * nh:(dc + 1) * nh, 0:D],
                                in_=ps.rearrange("p (h c) -> p h c", c=D),
                            )
                            nc.vector.memset(
                                qn4[:, st, dc * nh:(dc + 1) * nh, D:D + 1], 1.0
                            )

            pproj = ctx.enter_context(
                tc.tile_pool(name="pproj", bufs=2, space="PSUM"))
            psc = ctx.enter_context(
                tc.tile_pool(name="psc", bufs=2, space="PSUM"))
            pav = ctx.enter_context(
                tc.tile_pool(name="pav", bufs=1, space="PSUM"))

            # ---- per head pair: K^T proj, Q^T proj, then attention ----
            # Normalization work for chunk qc is EMITTED a few units into the
            # following chunk's stream: the PE reaches the broadcast matmul
            # long after its stg input is ready, so it never stalls, and the
            # pav banks were already freed by the eviction copies.
            pending = []

            def flush_pending(use_pav=False):
                for (php, pqc, stgs) in pending:
                    for half in range(2):
                        if use_pav:
                            rb = pav.tile(
                                [P, CH], f32,
                                tag=("pvA" if half == 0 else "pvB"), name="rb",
                            )
                        else:
                            rb = pproj.tile([P, CH], f32, tag="ps", name="rb")
                        nc.tensor.matmul(
                            rb, ones128[0:1, :], stgs[half],
                            start=True, stop=True,
                        )
                        rcp = stgp.tile([P, CH], f32, tag="rbs", bufs=1)
                        nc.vector.reciprocal_approx_fast(
                            out=rcp, in_=rb
                        )
                        dst = attnT[
                            half * D:(half + 1) * D,
                            php,
                            pqc * CH:(pqc + 1) * CH,
                        ]
                        nc.vector.tensor_tensor(
                            out=dst, in0=dst,
                            in1=rcp[half * D:(half + 1) * D, :],
                            op=mybir.AluOpType.mult,
                        )
                pending.clear()

            # Projections run one head pair AHEAD of attention, with their
            # six PSUM groups interleaved between attention units so neither
            # the PE nor the eviction engines ever stall on each other.
            def issue_whp_dmas(hp):
                wk_hp = whp.tile([P, E_t, P], bf16, tag="wk_hp", name="wk_hp")
                wq_hp = whp.tile([P, E_t, P], bf16, tag="wq_hp", name="wq_hp")
                for e in range(E_t):
                    nc.sync.dma_start(
                        out=wk_hp[:, e, :],
                        in_=wk_r[:, e, hp * P:(hp + 1) * P],
                    )
                    nc.sync.dma_start(
                        out=wq_hp[:, e, :],
                        in_=wq_r[:, e, hp * P:(hp + 1) * P],
                    )
                return wk_hp, wq_hp

            def proj_tasks(hp, wk_hp, wq_hp):
                tasks = []

                def k_group(sc, hp=hp, wk_hp=wk_hp):
                    ps = pproj.tile([P, SCH], f32, tag="ps", name="ps")
                    for e in range(E_t):
                        nc.tensor.matmul(
                            ps,
                            wk_hp[:, e, :],
                            xTs[e][:, sc * SCH:(sc + 1) * SCH],
                            start=(e == 0),
                            stop=(e == E_t - 1),
                        )
                    nc.scalar.copy(
                        out=kT[:, hp, sc * SCH:(sc + 1) * SCH], in_=ps
                    )

                def q_group(qc2, hp=hp, wq_hp=wq_hp):
                    ps = pproj.tile([P, QCH], f32, tag="ps", name="ps")
                    for e in range(E_t):
                        nc.tensor.matmul(
                            ps,
                            wq_hp[:, e, :],
                            xqTs[e][:, qc2 * QCH:(qc2 + 1) * QCH],
                            start=(e == 0),
                            stop=(e == E_t - 1),
                        )
                    nc.scalar.copy(
                        out=qT[:, hp, qc2 * QCH:(qc2 + 1) * QCH], in_=ps
                    )

                for sc in range(S // SCH):
                    tasks.append(lambda sc=sc: k_group(sc))
                for qc2 in range(Lq // QCH):
                    tasks.append(lambda qc2=qc2: q_group(qc2))
                return tasks

            if True:
                wk0, wq0 = issue_whp_dmas(0)
                for g in proj_tasks(0, wk0, wq0):
                    g()
                for hp in range(HP):
                    hA, hB = 2 * hp, 2 * hp + 1
                    tasks = []
                    if hp + 1 < HP:
                        wkn, wqn = issue_whp_dmas(hp + 1)
                        tasks = proj_tasks(hp + 1, wkn, wqn)
                    ti = 0
                    ucount = 0

                    # ---- attention for this head pair ----
                    for qc in range(n_ch):
                        pvA = pav.tile([P, CH], f32)
                        pvB = pav.tile([P, CH], f32)
                        t_max = 2 * (qc * spc + spc - 1) + 1
                        for t in range(t_max + 1):
                            if t == 2:
                                flush_pending()
                            jloc = uofs[(qc, t)]
                            qoff = jloc * P
                            sc_t = psc.tile([P, 2, CHB], f32)
                            nc.tensor.matmul(
                                sc_t[:, 0, qoff:CH],
                                kT[0:D, hp, t * P:(t + 1) * P],
                                qT[0:D, hp, qc * CH + qoff:(qc + 1) * CH],
                                start=True,
                                stop=True,
                            )
                            nc.tensor.matmul(
                                sc_t[:, 1, qoff:CH],
                                kT[D:P, hp, t * P:(t + 1) * P],
                                qT[D:P, hp, qc * CH + qoff:(qc + 1) * CH],
                                start=True,
                                stop=True,
                            )
                            ex = expp.tile([P, 2, CH], bf16)
                            nc.scalar.activation(
                                out=ex[:, :, qoff:CH],
                                in_=sc_t[:, :, qoff:CH],
                                func=Exp,
                                scale=scale,
                            )
                            if (qc, t) in mu:
                                mi = mu[(qc, t)]
                                for h2 in range(2):
                                    nc.vector.tensor_mul(
                                        out=ex[:, h2, qoff:qoff + P],
                                        in0=ex[:, h2, qoff:qoff + P],
                                        in1=masks[:, mi, :],
                                    )
                            if ((qc == 0 and t >= 5) or
                                    (qc == 1 and t < 3)) and ti < len(tasks):
                                tasks[ti]()
                                ti += 1
                            nc.tensor.matmul(
                                pvA[0:D + 1, qoff:CH],
                                qn[:, t, hA * (D + 1):(hA + 1) * (D + 1)],
                                ex[:, 0, qoff:CH],
                                start=(t == 0),
                                stop=(t == t_max),
                            )
                            nc.tensor.matmul(
                                pvB[0:D + 1, qoff:CH],
                                qn[:, t, hB * (D + 1):(hB + 1) * (D + 1)],
                                ex[:, 1, qoff:CH],
                                start=(t == 0),
                                stop=(t == t_max),
                            )
                        # evict unnormalized attn + rowsum row, then queue the
                        # normalization for emission inside the next chunk.
                        stgs = []
                        for pv, half in ((pvA, 0), (pvB, 1)):
                            stg = stgp.tile([1, CH], bf16, tag="stg", bufs=2)
                            nc.vector.tensor_copy(out=stg, in_=pv[D:D + 1, :])
                            nc.vector.tensor_copy(
                                out=attnT[
                                    half * D:(half + 1) * D,
                                    hp,
                                    qc * CH:(qc + 1) * CH,
                                ],
                                in_=pv[0:D, :],
                            )
                            stgs.append(stg)
                        pending.append((hp, qc, stgs))
                    while ti < len(tasks):
                        tasks[ti]()
                        ti += 1

        # keep the PE warm across the attention->outproj transition
        wdst2 = psc.tile([P, 2, CHB], f32, tag="sc_t")
        for _ in range(4):
            nc.tensor.matmul(
                wdst2[:, 0, 0:256], wsrc[:, 0:P], wsrc, start=True, stop=True
            )
        flush_pending(use_pav=True)

        # ---- output projection + bias ----
        if True:
            for st in range(nq):
                for ec in range(E // ECH):
                    ps = pproj.tile([P, ECH], f32, tag="ps")
                    nc.tensor.matmul(
                        ps,
                        ones128[0:1, :],
                        bo_sb[0:1, ec * ECH:(ec + 1) * ECH],
                        start=True,
                        stop=False,
                    )
                    for cp in range(HP):
                        nc.tensor.matmul(
                            ps,
                            attnT[:, cp, st * P:(st + 1) * P],
                            wo[:, cp, ec * ECH:(ec + 1) * ECH],
                            start=False,
                            stop=(cp == HP - 1),
                        )
                    ot = ostp.tile([P, ECH], bf16, bufs=3)
                    nc.vector.tensor_copy(out=ot, in_=ps)
                    nc.sync.dma_start(
                        out=out_d[st * P:(st + 1) * P, ec * ECH:(ec + 1) * ECH],
                        in_=ot,
                    )

    nc.finalize()
    return nc


def _host_masks(S, parity, P=128):
    """Frontier masks for mask-applied units, partition-major [P, U2*P] bf16."""
    units, CH, spc, n_ch = _unit_list(S, P)
    mus = _masked_units(S, P)
    tri = np.triu(np.ones((P, P), dtype=np.float32))
    ones = np.ones((P, P), dtype=np.float32)
    zeros = np.zeros((P, P), dtype=np.float32)
    out = np.empty((len(mus), P, P), dtype=np.float32)
    for (qc, t, jloc) in units:
        if (qc, t) not in mus:
            continue
        mi = mus[(qc, t)]
        g = 2 * (qc * spc + jloc) + parity
        if t < g:
            out[mi] = ones
        elif t == g:
            out[mi] = tri
        else:
            out[mi] = zeros
    # -> [P(k), U2, P(q)] -> flatten to [P, U2*P]
    pm = np.ascontiguousarray(out.transpose(1, 0, 2)).reshape(P, len(mus) * P)
    return pm.astype(BF16)


def _prep_inputs(x, Wk, Wq, Wo, bo, n_cores=NCORES):
    """Build per-core input maps (bf16; masks per parity)."""
    b, s, e = x.shape
    wqT = np.ascontiguousarray(Wq.T).astype(BF16)
    wkT = np.ascontiguousarray(Wk.T).astype(BF16)
    woT = np.ascontiguousarray(Wo.T).astype(BF16)
    bo2 = bo.reshape(1, e).astype(BF16)
    masks = [_host_masks(s, p) for p in (0, 1)]
    P = 128
    in_maps = []
    for c in range(n_cores):
        bi, p = c // 2, c % 2
        xb = x[bi]  # [S, E] f32
        xT = np.ascontiguousarray(xb.T).astype(BF16)
        qsel = xb.reshape(s // P, P, e)[p::2].reshape(s // 2, e)
        xqT = np.ascontiguousarray(qsel.T).astype(BF16)
        in_maps.append(
            {
                "xT": xT,
                "xqT": xqT,
                "wqT": wqT,
                "wkT": wkT,
                "woT": woT,
                "bo": bo2,
                "masks": masks[p],
            }
        )
    return in_maps


def kernel(x, Wk, Wq, Wv, Wo, bo):
    from concourse import bass_utils

    x = np.asarray(x, dtype=np.float32)
    Wk = np.asarray(Wk, dtype=np.float32)
    Wq = np.asarray(Wq, dtype=np.float32)
    Wo = np.asarray(Wo, dtype=np.float32)
    bo = np.asarray(bo, dtype=np.float32)
    b, s, e = x.shape
    h = H
    key = (s, e, h)
    if key not in _CACHE:
        _CACHE[key] = _build_program(s, e, h)
    nc = _CACHE[key]
    in_maps = _prep_inputs(x, Wk, Wq, Wo, bo)
    res = bass_utils.run_bass_kernel_spmd(nc, in_maps, list(range(NCORES)))
    P = 128
    out = np.empty((b, s, e), dtype=np.float32)
    for c in range(NCORES):
        bi, p = c // 2, c % 2
        oc = np.asarray(res.results[c]["out"], dtype=np.float32)  # [Lq, E]
        out[bi].reshape(s // P, P, e)[p::2] = oc.reshape(s // 2 // P, P, e)
    return out


if __name__ == "__main__":
    # smoke: build program only
    nc = _build_program(S, E, H)
    print("built ok")

